# revision 1
# baseline (speedup 1.0000x reference)
"""Trainium2 Bass kernel for the 3-view attention-fusion pooling module.

Computation (reference):
    t_k  = tanh(W @ x_k)                      (A=256, D=256), k = 1..3
    s_k  = h_n @ t_k                          (1, D)
    beta = softmax([s_1; s_2; s_3], axis=0)   (3, D)
    out  = beta[0]*x1 + beta[1]*x2 + beta[2]*x3   (N, D)

Sharding: rows (node dim N=100000) split evenly across 8 cores. W is fed
per-core as W[:, shard].T (host-side transpose) so the contraction dim is
the partition dim for the TensorE matmul. The (A, D) GEMM partials are
AllReduce-summed across cores; everything downstream of the reduction is
tiny and computed redundantly on every core.

Layout: within a batch of P*R rows, partition p holds R consecutive DRAM
rows (p*R .. p*R+R-1) so every DMA moves R*D*4 contiguous bytes per
partition. The GEMM contraction is order-invariant, and x / W^T / out all
use the same row->(p,r) mapping, so the permutation cancels out.

The last ST batches of x stay resident in SBUF after phase 1 (stash) and
are consumed directly by phase 2, skipping their reload from HBM.
"""

import sys

import numpy as np

for _p in ("/opt/trn_rl_repo", "/root/.axon_site/_ro/trn_rl_repo"):
    if _p not in sys.path:
        sys.path.append(_p)

import concourse.bacc as bacc
import concourse.tile as tile
from concourse import mybir
from concourse.bass_utils import run_bass_kernel_spmd

N_CORES = 8
N = 100000
D = 256          # feature dim
A = 256          # input_att
N_LOC = N // N_CORES   # 12500 rows per core
P = 125          # partitions per batch (matmul contraction chunk)
R = 5            # rows per partition per batch
NB = N_LOC // (P * R)  # 20 batches
FW = R * D       # free width of a batched SBUF tile
ST = 7           # batches of x stashed in SBUF between the passes
PX_BUFS = 3      # streaming x pool depth

FP32 = mybir.dt.float32
MM_DT = mybir.dt.float32r  # matmul compute dtype (same storage as fp32)


def _emit_iteration(nc, tc, rep, xrs, wtr, outr, hn_sb, ones_sb, pdram,
                    n_cores, collective, phase2=True):
    Tanh = mybir.ActivationFunctionType.Tanh
    Exp = mybir.ActivationFunctionType.Exp
    n_stream = NB - ST
    r = rep

    with (
        tc.tile_pool(name=f"px1_{r}", bufs=PX_BUFS) as px1,
        tc.tile_pool(name=f"px2_{r}", bufs=PX_BUFS) as px2,
        tc.tile_pool(name=f"px3_{r}", bufs=PX_BUFS) as px3,
        tc.tile_pool(name=f"pst1_{r}", bufs=ST) as pst1,
        tc.tile_pool(name=f"pst2_{r}", bufs=ST) as pst2,
        tc.tile_pool(name=f"pst3_{r}", bufs=ST) as pst3,
        tc.tile_pool(name=f"small_{r}", bufs=1) as small,
    ):
        xpools = (px1, px2, px3)
        stpools = (pst1, pst2, pst3)

        # ---------------- phase 1: u_k = W @ x_k (per-core partials) -------
        stash = {}
        cc_in = small.tile([128, 6 * D], FP32, name="cc_in", tag="cc_in")
        with (
            tc.tile_pool(name=f"pacc_{r}", bufs=1, space="PSUM") as pacc,
            tc.tile_pool(name=f"pw_{r}", bufs=4) as pw,
        ):
            uacc = [[pacc.tile([128, D], FP32, name=f"u{v}{h}",
                               tag=f"u{v}{h}")
                     for h in range(2)] for v in range(3)]
            for b in range(NB):
                stashed = b >= n_stream
                xts = []
                for v in range(3):
                    pool = stpools[v] if stashed else xpools[v]
                    t = pool.tile([P, FW], MM_DT,
                                  name=f"{'xs' if stashed else 'x'}{v}",
                                  tag="xs" if stashed else "x")
                    # split loads across both HWDGE rings (SP + ACT)
                    eng = nc.sync if v < 2 else nc.scalar
                    eng.dma_start(t[:], xrs[v][b].bitcast(MM_DT))
                    xts.append(t)
                if stashed:
                    stash[b] = xts
                wtile = pw.tile([P, FW], MM_DT, name="w", tag="w")
                nc.scalar.dma_start(wtile[:], wtr[b].bitcast(MM_DT))
                for g in range(R):
                    first = (b == 0 and g == 0)
                    last = (b == NB - 1 and g == R - 1)
                    for h in range(2):
                        lhs = wtile[:, g * A + h * 128: g * A + h * 128 + 128]
                        for v in range(3):
                            nc.tensor.matmul(
                                uacc[v][h][:],
                                lhsT=lhs,
                                rhs=xts[v][:, g * D:(g + 1) * D],
                                start=first, stop=last)
            for v in range(3):
                for h in range(2):
                    i = v * 2 + h
                    nc.vector.tensor_copy(cc_in[:, i * D:(i + 1) * D],
                                          uacc[v][h][:])

        # ---------------- all-reduce the GEMM partials ----------------------
        ccin_d = pdram.tile([128, 6 * D], FP32, name=f"ccin{r}",
                            tag=f"ccin{r}")
        ccout_d = pdram.tile([128, 6 * D], FP32, name=f"ccout{r}",
                             tag=f"ccout{r}")
        nc.sync.dma_start(ccin_d[:], cc_in[:])
        if collective:
            nc.gpsimd.collective_compute(
                "AllReduce", mybir.AluOpType.add,
                replica_groups=[list(range(n_cores))],
                ins=[ccin_d.opt()], outs=[ccout_d.opt()])
        else:
            nc.sync.dma_start(ccout_d[:], ccin_d[:])
        # reuse cc_in for the reduced result; tanh in place
        t_tanh = cc_in
        nc.sync.dma_start(t_tanh[:], ccout_d[:])

        # ---------------- tanh, scores, softmax, beta broadcast -------------
        nc.scalar.activation(t_tanh[:], t_tanh[:], Tanh)

        evs = []
        Bsb = []
        with (
            tc.tile_pool(name=f"ps_{r}", bufs=1, space="PSUM") as ps,
            tc.tile_pool(name=f"pB_{r}", bufs=1, space="PSUM") as pB,
        ):
            for v in range(3):
                s_ps = ps.tile([1, D], FP32, name=f"s{v}", tag=f"s{v}")
                for h in range(2):
                    i = v * 2 + h
                    nc.tensor.matmul(
                        s_ps[:], lhsT=hn_sb[:, h:h + 1],
                        rhs=t_tanh[:, i * D:(i + 1) * D],
                        start=(h == 0), stop=(h == 1))
                e_v = small.tile([1, D], FP32, name=f"e{v}", tag=f"e{v}")
                nc.scalar.activation(e_v[:], s_ps[:], Exp)
                evs.append(e_v)
            ssum = small.tile([1, D], FP32, name="ssum", tag="ssum")
            nc.vector.tensor_add(ssum[:], evs[0][:], evs[1][:])
            nc.vector.tensor_add(ssum[:], ssum[:], evs[2][:])
            rinv = small.tile([1, D], FP32, name="rinv", tag="rinv")
            nc.vector.reciprocal(rinv[:], ssum[:])
            for v in range(3):
                b_v = small.tile([1, D], FP32, name=f"bt{v}", tag=f"bt{v}")
                nc.vector.tensor_mul(b_v[:], evs[v][:], rinv[:])
                B_ps = pB.tile([128, D], FP32, name=f"B{v}", tag=f"B{v}")
                nc.tensor.matmul(B_ps[:], lhsT=ones_sb[:], rhs=b_v[:],
                                 start=True, stop=True)
                B_v = small.tile([128, D], FP32, name=f"Bb{v}", tag=f"Bb{v}")
                nc.vector.tensor_copy(B_v[:], B_ps[:])
                Bsb.append(B_v)

        # ---------------- phase 2: out = sum_k beta_k * x_k -----------------
        if not phase2:
            # timing-decomposition variant: skip phase 2, emit a token store
            nc.sync.dma_start(outr[0][:, 0:6 * D], t_tanh[:])
            return
        Bb = [Bsb[v][0:P, :].unsqueeze(1).broadcast_to([P, R, D])
              for v in range(3)]
        with (
            tc.tile_pool(name=f"pout_{r}", bufs=2) as pout,
            tc.tile_pool(name=f"ptmp_{r}", bufs=2) as ptmp,
        ):
            # interleave stashed and streamed batches: compute on stashed
            # (already-resident) data overlaps the streamed reloads, and the
            # first batches need no DMA so compute starts right after beta.
            stashed_l = list(range(n_stream, NB))
            streamed_l = list(range(n_stream))
            order = []
            while stashed_l or streamed_l:
                if stashed_l:
                    order.append(stashed_l.pop(0))
                if streamed_l:
                    order.append(streamed_l.pop(0))
            for b in order:
                if b in stash:
                    xts = stash[b]
                    xs3 = [t[:].bitcast(FP32)
                           .rearrange("p (r d) -> p r d", r=R)
                           for t in xts]
                else:
                    xts = []
                    for v in range(3):
                        t = xpools[v].tile([P, FW], FP32, name=f"x{v}",
                                           tag="x")
                        eng = nc.sync if v < 2 else nc.scalar
                        eng.dma_start(t[:], xrs[v][b])
                        xts.append(t)
                    xs3 = [t[:].rearrange("p (r d) -> p r d", r=R)
                           for t in xts]
                ot = pout.tile([P, FW], FP32, name="o", tag="o")
                ta = ptmp.tile([P, FW], FP32, name="ta", tag="ta")
                tb = ptmp.tile([P, FW], FP32, name="tb", tag="tb")
                o3 = ot[:].rearrange("p (r d) -> p r d", r=R)
                ta3 = ta[:].rearrange("p (r d) -> p r d", r=R)
                tb3 = tb[:].rearrange("p (r d) -> p r d", r=R)
                # split the 3 muls across DVE and GpSimd (independent), then
                # chain the adds on DVE: dependency depth 3 instead of 5
                nc.vector.tensor_mul(o3, xs3[0], Bb[0])
                nc.gpsimd.tensor_mul(ta3, xs3[1], Bb[1])
                nc.gpsimd.tensor_mul(tb3, xs3[2], Bb[2])
                nc.vector.tensor_add(o3, o3, ta3)
                nc.vector.tensor_add(o3, o3, tb3)
                nc.scalar.dma_start(outr[b], ot[:])


def build_bass(n_cores=N_CORES, collective=True, repeat=1, phase2=True):
    nc = bacc.Bacc("TRN2", target_bir_lowering=False, debug=False,
                   num_devices=n_cores)

    x1 = nc.dram_tensor("x1", [N_LOC, D], FP32, kind="ExternalInput")
    x2 = nc.dram_tensor("x2", [N_LOC, D], FP32, kind="ExternalInput")
    x3 = nc.dram_tensor("x3", [N_LOC, D], FP32, kind="ExternalInput")
    wt = nc.dram_tensor("wt", [N_LOC, A], FP32, kind="ExternalInput")
    hnt = nc.dram_tensor("hnt", [A, 1], FP32, kind="ExternalInput")
    out = nc.dram_tensor("out", [N_LOC, D], FP32, kind="ExternalOutput")

    with tile.TileContext(nc) as tc:
        with (
            tc.tile_pool(name="smallg", bufs=1) as smallg,
            tc.tile_pool(name="pdram", bufs=1, space="DRAM") as pdram,
        ):
            x1r = x1.ap().rearrange("(b p r) d -> b p (r d)", p=P, r=R)
            x2r = x2.ap().rearrange("(b p r) d -> b p (r d)", p=P, r=R)
            x3r = x3.ap().rearrange("(b p r) d -> b p (r d)", p=P, r=R)
            wtr = wt.ap().rearrange("(b p r) a -> b p (r a)", p=P, r=R)
            outr = out.ap().rearrange("(b p r) d -> b p (r d)", p=P, r=R)
            xrs = (x1r, x2r, x3r)

            # h_n laid out [a_half(128 partitions), h(2)]
            hn_sb = smallg.tile([128, 2], FP32, tag="hn")
            nc.sync.dma_start(hn_sb[:, :],
                              hnt.ap().rearrange("(h a) o -> a (h o)", h=2))
            ones_sb = smallg.tile([1, 128], FP32, tag="ones")
            nc.vector.memset(ones_sb[:], 1.0)

            for rep in range(repeat):
                _emit_iteration(nc, tc, rep, xrs, wtr, outr, hn_sb, ones_sb,
                                pdram, n_cores, collective, phase2)

    nc.compile()
    return nc


_NC_CACHE = {}


def _get_nc():
    if "nc" not in _NC_CACHE:
        _NC_CACHE["nc"] = build_bass()
    return _NC_CACHE["nc"]


def kernel(x1, x2, x3, W, h_n):
    x1 = np.ascontiguousarray(x1, dtype=np.float32)
    x2 = np.ascontiguousarray(x2, dtype=np.float32)
    x3 = np.ascontiguousarray(x3, dtype=np.float32)
    W = np.ascontiguousarray(W, dtype=np.float32)
    h_n = np.ascontiguousarray(h_n, dtype=np.float32)

    hnt = np.ascontiguousarray(h_n.reshape(-1)[:, None])  # (A, 1)
    in_maps = []
    for c in range(N_CORES):
        sl = slice(c * N_LOC, (c + 1) * N_LOC)
        in_maps.append({
            "x1": x1[sl],
            "x2": x2[sl],
            "x3": x3[sl],
            "wt": np.ascontiguousarray(W[:, sl].T),
            "hnt": hnt,
        })

    nc = _get_nc()
    res = run_bass_kernel_spmd(nc, in_maps, core_ids=list(range(N_CORES)))
    return np.concatenate([res.results[c]["out"] for c in range(N_CORES)],
                          axis=0)



# revision 38
# speedup vs baseline: 15.7278x; 15.7278x over previous
"""Trainium2 Bass kernel for the 3-view attention-fusion pooling module.

Computation (reference):
    t_k  = tanh(W @ x_k)                      (A=256, D=256), k = 1..3
    s_k  = h_n @ t_k                          (1, D)
    beta = softmax([s_1; s_2; s_3], axis=0)   (3, D)
    out  = beta[0]*x1 + beta[1]*x2 + beta[2]*x3   (N, D)

Sharding: rows (node dim N=100000) split evenly across 8 cores; W fed
per-core as W[:, shard].T. The (A, 3D) GEMM partials are AllReduce-summed
across cores; everything downstream of the reduction is tiny and computed
redundantly on every core.

v3 datapath (rel-err budget is 2e-2; measured ~5.5e-4 with this design):
  - host casts x1/x2/x3/W to fp16; device reads fp16 (halves HBM read)
  - GEMM in fp16 (fp32 PSUM accumulate), 1 PE cycle/row
  - batches NS.. of x stay resident in SBUF as fp16; phase 2 reads the
    stash, no reload. Batches 0..NS-1 are streamed twice: their loads and
    GEMM for iteration r+1 proceed while iteration r is still inside its
    collective+softmax bubble.
  - AllReduce payload fp16 (393 KiB), output in Shared pair-HBM
  - phase 2: out = b1*x1 + b2*x2 + b3*x3 per batch in fp16: DVE takes 4
    ops (2x fp16 mode), Pool takes 1; out stored fp16, host upcasts
  - software-pipelined emission across repeat iterations with GLOBAL tile
    pools: phase2(r) interleaves with phase1(r+1) batch-by-batch in every
    engine's program order, so in-order queues never head-of-line block
    the next iteration. Out-stores go through the Pool SWDGE queue so the
    SP/ACT HWDGE rings only carry loads.

Layout: within a batch of P*R rows, partition p holds R consecutive DRAM
rows, so every DMA moves R*D*2 contiguous bytes per partition. The GEMM
contraction is order-invariant and x / W^T / out share the row mapping,
so the permutation cancels.
"""

import sys

import numpy as np

for _p in ("/opt/trn_rl_repo", "/root/.axon_site/_ro/trn_rl_repo"):
    if _p not in sys.path:
        sys.path.append(_p)

import concourse.bacc as bacc
import concourse.tile as tile
from concourse import mybir
from concourse.bass_utils import run_bass_kernel_spmd

N_CORES = 8
N = 100000
D = 256          # feature dim
A = 256          # input_att
N_LOC = N // N_CORES   # 12500 rows per core
P = 125          # partitions per batch (matmul contraction chunk)
R = 5            # rows per partition per batch
NB = N_LOC // (P * R)  # 20 batches
FW = R * D       # free width of a batched SBUF tile (elements)
NS = 6           # batches streamed twice rather than stashed: their
                 # loads+GEMM for iteration r+1 fill the collective
                 # window of iteration r

FP32 = mybir.dt.float32
FP16 = mybir.dt.float16

Tanh = mybir.ActivationFunctionType.Tanh
Exp = mybir.ActivationFunctionType.Exp
Copy = mybir.ActivationFunctionType.Copy


class _Pools:
    pass


def _mk_pools(tc, ctx):
    p = _Pools()
    ent = ctx.enter_context
    p.pst = [ent(tc.tile_pool(name=f"pst{v}", bufs=NB - NS))
             for v in range(3)]
    p.pss = ent(tc.tile_pool(name="pss", bufs=12))
    p.pw = ent(tc.tile_pool(name="pw", bufs=2))
    p.small = ent(tc.tile_pool(name="small", bufs=1))
    p.pcc = ent(tc.tile_pool(name="pcc", bufs=2))
    p.pbeta = ent(tc.tile_pool(name="pbeta", bufs=2))
    p.pout = ent(tc.tile_pool(name="pout", bufs=3))
    p.ptmp = ent(tc.tile_pool(name="ptmp", bufs=2))
    p.pm2 = ent(tc.tile_pool(name="pm2", bufs=2))
    p.pacc = ent(tc.tile_pool(name="pacc", bufs=1, space="PSUM"))
    p.psc = ent(tc.tile_pool(name="psc", bufs=1, space="PSUM"))
    p.pdram = ent(tc.tile_pool(name="pdram", bufs=2, space="DRAM"))
    return p


def _load_batch(nc, p, xrs, b, streamed):
    """DMA the three x views of batch b; x1,x3 on the SP ring, x2 on ACT."""
    xeng = (nc.sync, nc.scalar, nc.sync)
    xts = []
    for v in range(3):
        pool = p.pss if streamed else p.pst[v]
        t = pool.tile([P, FW], FP16,
                      name=f"{'xq' if streamed else 'xs'}{v}",
                      tag="xq" if streamed else f"xs{v}")
        xeng[v].dma_start(t[:], xrs[v][b])
        xts.append(t)
    return xts


def _gemm_batch(nc, uacc, wtile, xts, b):
    """uacc[v][h] each own a full 2 KiB PSUM bank: the six accumulation
    groups stay open across the whole GEMM, and a start=True zeroes its
    entire bank — two open groups must never share one."""
    for g in range(R):
        first = (b == 0 and g == 0)
        last = (b == NB - 1 and g == R - 1)
        for h in range(2):
            lhs = wtile[:, g * A + h * 128: g * A + h * 128 + 128]
            for v in range(3):
                nc.tensor.matmul(
                    uacc[v][h][:, 0:D], lhsT=lhs,
                    rhs=xts[v][:, g * D:(g + 1) * D],
                    start=first, stop=last)


def _phase2_batch(nc, p, outr, Bsb, xts, b):
    """out[b] = B1*x1 + B2*x2 + B3*x3; DVE 4 ops, Pool 1 op, store on the
    Pool SWDGE queue (keeps the HWDGE load rings free of stores)."""
    x1t, x2t, x3t = xts
    m2 = p.pm2.tile([P, FW], FP16, name="m2", tag="m2")
    nc.gpsimd.tensor_mul(m2[:], x2t[:], Bsb[1][0:P, :])
    m1 = p.ptmp.tile([P, FW], FP16, name="m1", tag="m1")
    m3 = p.ptmp.tile([P, FW], FP16, name="m3", tag="m3")
    ot = p.pout.tile([P, FW], FP16, name="o", tag="o")
    nc.vector.tensor_mul(m1[:], x1t[:], Bsb[0][0:P, :])
    nc.vector.tensor_mul(m3[:], x3t[:], Bsb[2][0:P, :])
    nc.vector.tensor_add(m1[:], m1[:], m2[:])
    nc.vector.tensor_add(ot[:], m1[:], m3[:])
    nc.gpsimd.dma_start(outr[b], ot[:])


def _cc_dispatch(nc, p, uacc, n_cores, collective, shared_cc, cc_dt):
    """Partials out of PSUM, then AllReduce. Every cc-related DMA stays on
    the Pool SWDGE queue: the cc_red load waits on the collective, and an
    in-order HWDGE queue would head-of-line block the next iteration's x
    loads behind it."""
    cc_in = p.pcc.tile([128, 6 * D], cc_dt, name="cc_in", tag="cc_in")
    for v in range(3):
        for h in range(2):
            i = v * 2 + h
            nc.scalar.activation(cc_in[:, i * D:(i + 1) * D],
                                 uacc[v][h][:, 0:D], Copy)
    ccin_d = p.pdram.tile([128, 6 * D], cc_dt, name="ccin", tag="ccin")
    ccout_d = p.pdram.tile([128, 6 * D], cc_dt, name="ccout", tag="ccout",
                           addr_space="Shared" if shared_cc else "Local")
    nc.gpsimd.dma_start(ccin_d[:], cc_in[:])
    if collective:
        nc.gpsimd.collective_compute(
            "AllReduce", mybir.AluOpType.add,
            replica_groups=[list(range(n_cores))],
            ins=[ccin_d.opt()], outs=[ccout_d.opt()])
    else:
        nc.gpsimd.dma_start(ccout_d[:], ccin_d[:])
    # cc_in is dead after the store; land the reduced result in it
    nc.gpsimd.dma_start(cc_in[:], ccout_d[:])
    return cc_in


def _beta_tail(nc, p, cc_red, hn_sb, ones_sb, cc_dt):
    """tanh -> scores -> softmax -> fp16 beta broadcast tiles [128, FW]
    (double-buffered across iterations). Emitted AFTER the next
    iteration's streamed GEMM so the PE queue is not head-of-line blocked
    on the collective."""
    t_tanh = p.small.tile([128, 6 * D], FP16, name="t_tanh", tag="t_tanh")
    nc.scalar.activation(t_tanh[:], cc_red[:], Tanh)

    evs = []
    Bsb = []
    # one full-bank scores tile, reused sequentially per view (the WAR
    # against the previous view's exp read orders the groups)
    s_ps = p.psc.tile([1, 512], FP32, name="s", tag="s")
    for v in range(3):
        sv = s_ps[:, 0:D]
        for h in range(2):
            i = v * 2 + h
            nc.tensor.matmul(
                sv, lhsT=hn_sb[:, h:h + 1],
                rhs=t_tanh[:, i * D:(i + 1) * D],
                start=(h == 0), stop=(h == 1))
        e_v = p.small.tile([1, D], FP32, name=f"e{v}", tag=f"e{v}")
        nc.scalar.activation(e_v[:], sv, Exp)
        evs.append(e_v)
    ssum = p.small.tile([1, D], FP32, name="ssum", tag="ssum")
    nc.vector.tensor_add(ssum[:], evs[0][:], evs[1][:])
    nc.vector.tensor_add(ssum[:], ssum[:], evs[2][:])
    rinv = p.small.tile([1, D], FP32, name="rinv", tag="rinv")
    nc.vector.reciprocal(rinv[:], ssum[:])
    B_ps = p.psc.tile([128, 512], FP32, name="Bps", tag="Bps")
    for v in range(3):
        b_v = p.small.tile([1, D], FP16, name=f"bt{v}", tag=f"bt{v}")
        nc.vector.tensor_mul(b_v[:], evs[v][:], rinv[:])
        nc.tensor.matmul(B_ps[:, 0:D], lhsT=ones_sb[:], rhs=b_v[:],
                         start=True, stop=True)
        B_v = p.pbeta.tile([128, FW], FP16, name=f"Bb{v}", tag=f"Bb{v}")
        # R-fold tile of beta along the free dim happens in this copy
        nc.scalar.activation(
            B_v[:].rearrange("p (r d) -> p r d", r=R),
            B_ps[:, 0:D].unsqueeze(1).broadcast_to([128, R, D]), Copy)
        Bsb.append(B_v)
    return Bsb


def build_bass(n_cores=N_CORES, collective=True, repeat=1, phase2=True,
               shared_cc=True, cc_dt=FP16):
    nc = bacc.Bacc("TRN2", target_bir_lowering=False, debug=False,
                   num_devices=n_cores)

    x1 = nc.dram_tensor("x1", [N_LOC, D], FP16, kind="ExternalInput")
    x2 = nc.dram_tensor("x2", [N_LOC, D], FP16, kind="ExternalInput")
    x3 = nc.dram_tensor("x3", [N_LOC, D], FP16, kind="ExternalInput")
    wt = nc.dram_tensor("wt", [N_LOC, A], FP16, kind="ExternalInput")
    hnt = nc.dram_tensor("hnt", [A, 1], FP32, kind="ExternalInput")
    out = nc.dram_tensor("out", [N_LOC, D], FP16, kind="ExternalOutput")

    from contextlib import ExitStack

    with tile.TileContext(nc) as tc, ExitStack() as ctx:
        p = _mk_pools(tc, ctx)
        x1r = x1.ap().rearrange("(b p r) d -> b p (r d)", p=P, r=R)
        x2r = x2.ap().rearrange("(b p r) d -> b p (r d)", p=P, r=R)
        x3r = x3.ap().rearrange("(b p r) d -> b p (r d)", p=P, r=R)
        wtr = wt.ap().rearrange("(b p r) a -> b p (r a)", p=P, r=R)
        outr = out.ap().rearrange("(b p r) d -> b p (r d)", p=P, r=R)
        xrs = (x1r, x2r, x3r)

        # h_n laid out [a_half(128 partitions), h(2)], cast to fp16 to
        # match the fp16 tanh tile in the scores matmul
        hn32 = p.small.tile([128, 2], FP32, name="hn32", tag="hn32")
        nc.sync.dma_start(hn32[:, :],
                          hnt.ap().rearrange("(h a) o -> a (h o)", h=2))
        hn_sb = p.small.tile([128, 2], FP16, name="hn", tag="hn")
        nc.vector.tensor_copy(hn_sb[:], hn32[:])
        ones_sb = p.small.tile([1, 128], FP16, name="ones", tag="ones")
        nc.vector.memset(ones_sb[:], 1.0)

        uacc = [[p.pacc.tile([128, 512], FP32, name=f"u{v}{h}",
                             tag=f"u{v}{h}")
                 for h in range(2)] for v in range(3)]

        cc_red = None   # un-reduced-yet cc result tile of iteration r-1
        pstash = None   # stash dict of iteration r-1
        for r in range(repeat):
            # streamed batches of r: their loads + GEMM fill the cc(r-1)
            # window (nothing here waits on the collective)
            for b in range(NS):
                wtile = p.pw.tile([P, R * A], FP16, name="w", tag="w")
                nc.scalar.dma_start(wtile[:], wtr[b])
                xts = _load_batch(nc, p, xrs, b, streamed=True)
                _gemm_batch(nc, uacc, wtile, xts, b)
            # beta tail of r-1 (first point that waits on cc(r-1))
            if cc_red is not None:
                pBsb = _beta_tail(nc, p, cc_red, hn_sb, ones_sb, cc_dt)
            # stash batches of r, interleaved with phase 2 of r-1
            stash = {}
            for b in range(NS, NB):
                if cc_red is not None and phase2:
                    _phase2_batch(nc, p, outr, pBsb, pstash[b], b)
                wtile = p.pw.tile([P, R * A], FP16, name="w", tag="w")
                nc.scalar.dma_start(wtile[:], wtr[b])
                xts = _load_batch(nc, p, xrs, b, streamed=False)
                stash[b] = xts
                _gemm_batch(nc, uacc, wtile, xts, b)
            # streamed batches of r-1: reload + phase 2
            if cc_red is not None and phase2:
                for b in range(NS):
                    xts = _load_batch(nc, p, xrs, b, streamed=True)
                    _phase2_batch(nc, p, outr, pBsb, xts, b)
            cc_red = _cc_dispatch(nc, p, uacc, n_cores, collective,
                                  shared_cc, cc_dt)
            pstash = stash

        # drain: last iteration's beta + phase 2
        pBsb = _beta_tail(nc, p, cc_red, hn_sb, ones_sb, cc_dt)
        if phase2:
            for b in range(NS, NB):
                _phase2_batch(nc, p, outr, pBsb, pstash[b], b)
            for b in range(NS):
                xts = _load_batch(nc, p, xrs, b, streamed=True)
                _phase2_batch(nc, p, outr, pBsb, xts, b)
        else:
            tok = p.pout.tile([P, FW], FP16, name="tok", tag="tok")
            nc.vector.tensor_mul(tok[:], pBsb[0][0:P, :],
                                 pBsb[1][0:P, :])
            nc.gpsimd.dma_start(outr[0], tok[:])

    nc.compile()
    return nc


_NC_CACHE = {}


def _get_nc():
    if "nc" not in _NC_CACHE:
        _NC_CACHE["nc"] = build_bass()
    return _NC_CACHE["nc"]


def kernel(x1, x2, x3, W, h_n):
    x1h = np.ascontiguousarray(x1, dtype=np.float16)
    x2h = np.ascontiguousarray(x2, dtype=np.float16)
    x3h = np.ascontiguousarray(x3, dtype=np.float16)
    Wh = np.ascontiguousarray(W, dtype=np.float16)
    h_n = np.ascontiguousarray(h_n, dtype=np.float32)

    hnt = np.ascontiguousarray(h_n.reshape(-1)[:, None])  # (A, 1)
    in_maps = []
    for c in range(N_CORES):
        sl = slice(c * N_LOC, (c + 1) * N_LOC)
        in_maps.append({
            "x1": x1h[sl],
            "x2": x2h[sl],
            "x3": x3h[sl],
            "wt": np.ascontiguousarray(Wh[:, sl].T),
            "hnt": hnt,
        })

    nc = _get_nc()
    res = run_bass_kernel_spmd(nc, in_maps, core_ids=list(range(N_CORES)))
    out16 = np.concatenate([res.results[c]["out"] for c in range(N_CORES)],
                           axis=0)
    return out16.astype(np.float32)
